# revision 10
# baseline (speedup 1.0000x reference)
"""Trainium2 Bass kernel for the scan-RNN problem (B=2048, T=512, H=256).

Data-parallel over batch: 8 cores x 256 rows each; the T=512 recurrence runs
fully on-chip per core with replicated (tiny) weights.

Math. Reference step:  z = (inp + h) @ W_update + b_update;  u = tanh(z);
h' = LN(u) * gamma + beta, with inp = tanh(x_t @ W_embed + b_embed) and
x_t integer in [0,10).  Host-side folding removes almost everything:

  * gamma/beta fold into the weights:  W' = diag(gamma) @ W_update,
    bp = b_update + beta @ W_update (and likewise for W_out).
  * the LN *mean subtraction* folds into the weights:
    ((u - mu) * r) @ W'  ==  (u * r) @ Wc   with  Wc = (I - J/H) @ W'
    (J = all-ones), because subtracting each column's mean from W'
    annihilates the per-row mean of the input exactly.
  * x_t has only 10 values, so the whole embedding path is a 10-row table
    LUT[v] = tanh(v*W_embed + b_embed) @ W_update + bp, applied per step as a
    K=10 one-hot matmul.

So the on-chip state is v = (u * rstd), kept TRANSPOSED (vT[h, b]) so each
step is:  psum_z = onehot.T @ LUT + vT.T @ Wc;  u = tanh(psum_z) (ACT, with
accumulated row-sum);  rstd via sum(u^2) (one scalar_tensor_tensor with
accum_out) + magic rsqrt seed + one Newton step (fused via the
RECIPROCAL_APPROX_NR custom DVE op);  vT' = PE-transpose of u with rhs = diag(rstd) (transpose-and-scale in
one matmul);  evacuate PSUM->SBUF (bf16 state).

Two independent 128-row batch chains per core are interleaved so the serial
per-step dependency chain of one chain hides under the other's engine work.
"""

import numpy as np

H = 256
EPS = 1e-5
NCORES = 8
NV = 10          # x values are 0..9
OHB = 16         # one-hot steps per DMA batch
N_NR = 1         # Newton iterations after the magic rsqrt seed
import os
USE_CUSTOM_NR = os.environ.get('NO_CUSTOM_NR', '') == ''
EVAC_ENGINE = os.environ.get('EVAC', 'dve,dve').split(',')
WARM_PE = os.environ.get('WARM_PE', '0') == '1'
USUM_DVE = os.environ.get('USUM', 'act') == 'dve'
EVSPLIT = os.environ.get('EVSPLIT', '0') == '1'
STATS_OFF = os.environ.get('STATS', 'on') == 'off'
# seed constant for rsqrt(2*vh) evaluated on bits of vh (0x5f3759df shifted
# down by 0.5 in the exponent to absorb the factor-2)
MAGIC = 0x5F3759DF - 0x00400000

# bf16 consts blob column layout ([128, _CWB] bf16)
_WC = 0              # Wc chunks: [128, 2, 256]
_GA = 512            # LUT (G_aug) [NV, 256] on partitions 0..9
_IM = 768            # identity mask [128, 128]
_WO = 896            # W_out' chunks [128, 2, 16]
_ONES = 928          # ones [1, 128] on partition 0
_OC = 1056           # out bias row [1, 16] on partition 0
_VT0 = 1072          # initial state vT [128, 2, 256]
_CWB = _VT0 + 512


def build_nc(T, B_local):
    """Build the Bass program for one core (SPMD: all cores identical)."""
    import concourse.bass as bass
    import concourse.mybir as mybir
    import concourse.tile as tile
    from concourse import bacc
    from concourse.dve_ops import RECIPROCAL_APPROX_NR

    dt = mybir.dt
    AF = mybir.ActivationFunctionType
    OP = mybir.AluOpType
    nc = bacc.Bacc(None, target_bir_lowering=False, debug=False)

    assert B_local == 256
    NB = 2  # batch chains of 128 rows

    ohb = min(OHB, T)
    assert T % ohb == 0
    oh = nc.declare_dram_parameter(
        "oh", [T // ohb, NV, ohb * B_local], dt.bfloat16, isOutput=False)
    cst = nc.declare_dram_parameter("cst", [128, _CWB], dt.bfloat16,
                                    isOutput=False)
    out = nc.declare_dram_parameter("out", [B_local, 16], dt.float32,
                                    isOutput=True)

    with tile.TileContext(nc) as tc:
        with (
            tc.tile_pool(name="singles", bufs=1) as singles,
            tc.tile_pool(name="ohpool", bufs=3) as ohpool,
            tc.tile_pool(name="state", bufs=3) as state,
            tc.tile_pool(name="upool", bufs=3) as upool,
            tc.tile_pool(name="scrp", bufs=3) as scrp,
            tc.tile_pool(name="stats", bufs=3) as stats,
            tc.tile_pool(name="diagp", bufs=3) as diagp,
            tc.tile_pool(name="psum_z", bufs=2, space="PSUM") as psum_z,
            tc.tile_pool(name="psum_t", bufs=2, space="PSUM") as psum_t,
        ):
            blob = singles.tile([128, _CWB], dt.bfloat16, tag="blob")
            nc.sync.dma_start(out=blob, in_=cst[:, :])
            wc = blob[:, _WC:_WC + 512].rearrange("p (k h) -> p k h", k=2)
            ga = blob[:NV, _GA:_GA + H]
            imask = blob[:, _IM:_IM + 128]
            wo = blob[:, _WO:_WO + 32].rearrange("p (k h) -> p k h", k=2)
            ones_row = blob[:1, _ONES:_ONES + 128]
            oc_row = blob[:1, _OC:_OC + 16]
            vt0 = blob[:, _VT0:_VT0 + 512].rearrange("p (k b) -> p k b", k=2)

            # per-chain transposed state [128(h), 2(chunk), 128(b)]
            vTs = [vt0[:, :, 0:128], vt0[:, :, 128:256]]

            oh_state = {"tile": None, "batch": -1}

            def emit_z_mm(t, c):
                """one-hot matmul opens the psum-z accumulation for (t, c)."""
                if oh_state["batch"] != t // ohb:
                    oh_bt = ohpool.tile([NV, ohb, B_local], dt.bfloat16,
                                        tag="oh")
                    nc.sync.dma_start(
                        out=oh_bt,
                        in_=oh[t // ohb, :, :].rearrange(
                            "v (s b) -> v s b", s=ohb),
                    )
                    oh_state["tile"] = oh_bt
                    oh_state["batch"] = t // ohb
                pz = psum_z.tile([128, H], dt.float32, tag=f"z{c}")
                nc.tensor.matmul(
                    pz, lhsT=oh_state["tile"][:, t % ohb, bass.ts(c, 128)],
                    rhs=ga, start=True, stop=False,
                )
                return pz

            pzs = None
            for t in range(T):
                if t == 0:
                    pzs = [emit_z_mm(0, 0), emit_z_mm(0, 1)]

                # ---- main matmuls (need prev state) ----------------------
                for c in range(NB):
                    for k in range(2):
                        nc.tensor.matmul(
                            pzs[c], lhsT=vTs[c][:, k, :], rhs=wc[:, k, :],
                            start=False, stop=(k == 1),
                        )

                us, dgs, new_vTs = [], [], []
                for c in range(NB):
                    st = stats.tile([128, 8], dt.float32, tag=f"s{c}")
                    usum = st[:, 0:1]
                    qsum = st[:, 1:2]
                    m2 = st[:, 2:3]
                    vh = st[:, 3:4]
                    yf = st[:, 4:5]
                    y0 = st[:, 5:6]
                    w = st[:, 6:7]
                    y1 = st[:, 7:8]

                    u = upool.tile([128, H], dt.bfloat16, tag=f"u{c}")
                    if USUM_DVE:
                        nc.scalar.activation(u, pzs[c], AF.Tanh)
                    else:
                        nc.scalar.activation(u, pzs[c], AF.Tanh,
                                             accum_out=usum)
                    us.append(u)

                    if STATS_OFF:
                        dgs.append(imask)
                        continue
                    # scr = (u * 0.5/H) * u ; qsum = sum(scr)  (one DVE op)
                    scr = scrp.tile([128, H], dt.bfloat16, tag=f"scr{c}")
                    if USUM_DVE:
                        nc.vector.tensor_scalar(
                            out=scr, in0=u, scalar1=1.0, scalar2=0.0,
                            op0=OP.mult, op1=OP.add, accum_out=usum,
                        )
                    nc.vector.scalar_tensor_tensor(
                        out=scr, in0=u, scalar=0.5 / H, in1=u,
                        op0=OP.mult, op1=OP.mult, accum_out=qsum,
                    )
                    # m2 = usum^2 - eps*H^2  (eps folded in so that vh gets +eps/2)
                    nc.vector.tensor_scalar(
                        out=m2, in0=usum, scalar1=usum, scalar2=-EPS * H * H,
                        op0=OP.mult, op1=OP.add,
                    )
                    # vh = 0.5*(var + eps) = qsum - 0.5/H^2 * m2
                    nc.vector.scalar_tensor_tensor(
                        out=vh, in0=m2, scalar=-0.5 / (H * H), in1=qsum,
                        op0=OP.mult, op1=OP.add,
                    )
                    # magic seed: y0 = bits_as_float(MAGIC - bits(vh)/2)
                    nc.vector.tensor_copy(out=yf, in_=vh.bitcast(dt.int32))
                    nc.vector.tensor_scalar(
                        out=yf, in0=yf, scalar1=-0.5, scalar2=float(MAGIC),
                        op0=OP.mult, op1=OP.add,
                    )
                    nc.vector.tensor_copy(out=y0.bitcast(dt.int32), in_=yf)
                    # Newton (fused): y' = (1.5 - (vh*y)*y)*y -> rsqrt(2*vh)
                    ys = [y0, y1, yf]  # yf slot is dead after the seed
                    for i in range(N_NR):
                        nc.vector.tensor_scalar_mul(w, ys[i], vh)
                        if USE_CUSTOM_NR:
                            nc.vector._custom_dve(
                                RECIPROCAL_APPROX_NR, out=ys[i + 1], in0=w,
                                in1=ys[i], s0=1.5,
                            )
                        else:
                            nc.vector.tensor_mul(w, w, ys[i])
                            nc.vector.tensor_scalar(
                                out=w, in0=w, scalar1=-1.0, scalar2=1.5,
                                op0=OP.mult, op1=OP.add,
                            )
                            nc.vector.tensor_mul(ys[i + 1], w, ys[i])
                    r = ys[N_NR]

                    dg = diagp.tile([128, 128], dt.bfloat16, tag=f"d{c}")
                    nc.vector.tensor_scalar_mul(dg, imask, r)
                    dgs.append(dg)

                # ---- prefetch next step's one-hot matmuls (fills the PE
                # stall while it waits on this step's diag) ----------------
                if t + 1 < T:
                    next_pzs = [emit_z_mm(t + 1, 0), emit_z_mm(t + 1, 1)]

                # ---- transpose-and-scale + per-chain psum evacuation -----
                for c in range(NB):
                    pt = psum_t.tile([128, 2, 128], dt.float32, tag=f"t{c}")
                    for k in range(2):
                        nc.tensor.matmul(
                            pt[:, k, :], lhsT=us[c][:, bass.ts(k, 128)],
                            rhs=dgs[c], start=True, stop=True,
                        )
                    vT = state.tile([128, 2, 128], dt.bfloat16, tag=f"vt{c}")
                    if EVAC_ENGINE[c] == "act":
                        nc.scalar.copy(out=vT, in_=pt)
                    elif EVSPLIT:
                        nc.vector.tensor_copy(out=vT[:, 0, :], in_=pt[:, 0, :])
                        nc.vector.tensor_copy(out=vT[:, 1, :], in_=pt[:, 1, :])
                    else:
                        nc.vector.tensor_copy(out=vT, in_=pt)
                    new_vTs.append(vT)
                vTs = new_vTs
                if t + 1 < T:
                    pzs = next_pzs

            # ---- final projection: out = vT @ Wo_c + oc ------------------
            po = psum_t.tile([128, 2, 16], dt.float32, tag="t0")
            for c in range(NB):
                for k in range(2):
                    nc.tensor.matmul(
                        po[:, c, :], lhsT=vTs[c][:, k, :], rhs=wo[:, k, :],
                        start=(k == 0), stop=False,
                    )
                nc.tensor.matmul(
                    po[:, c, :], lhsT=ones_row, rhs=oc_row,
                    start=False, stop=True,
                )
            ot = upool.tile([128, 2, 16], dt.float32, tag="ot")
            nc.vector.tensor_copy(out=ot, in_=po)
            nc.sync.dma_start(
                out=out[:, :].rearrange("(c p) h -> p c h", p=128), in_=ot
            )

    nc.finalize()
    return nc


def _bf16(a):
    import ml_dtypes
    return np.asarray(a, dtype=ml_dtypes.bfloat16)


def _prepare_host(x, W_embed, b_embed, W_update, b_update, gamma, beta,
                  W_out, b_out):
    """Fold gamma/beta + LN mean-centering into weights; build one-hot."""
    Wp = (gamma[:, None] * W_update).astype(np.float32)          # [H, H]
    Wc = Wp - Wp.mean(axis=0, keepdims=True)                     # center cols
    bp = (b_update + beta @ W_update).astype(np.float32)
    vals = np.arange(NV, dtype=np.float32)[:, None]
    LUT = (np.tanh(vals @ W_embed + b_embed) @ W_update + bp).astype(
        np.float32)                                              # [NV, H]
    Wo = (gamma[:, None] * W_out).astype(np.float32)             # [H, 10]
    Wo_c = Wo - Wo.mean(axis=0, keepdims=True)
    oc = (b_out + beta @ W_out).astype(np.float32)               # [10]
    hhat0 = (-beta / gamma).astype(np.float32)                   # [H]

    xi = x[:, :, 0].astype(np.int32)                             # [B, T]
    B, T = xi.shape
    oh = np.zeros((T, NV, B), np.float32)
    tidx = np.broadcast_to(np.arange(T)[:, None], (T, B))
    bidx = np.broadcast_to(np.arange(B)[None, :], (T, B))
    oh[tidx, xi.T, bidx] = 1.0

    cst = np.zeros((128, _CWB), np.float32)
    cst[:, _WC + 0:_WC + 256] = Wc[0:128]
    cst[:, _WC + 256:_WC + 512] = Wc[128:256]
    cst[:NV, _GA:_GA + H] = LUT
    cst[:, _IM:_IM + 128] = np.eye(128, dtype=np.float32)
    cst[:, _WO:_WO + 16] = np.pad(Wo_c[0:128], ((0, 0), (0, 6)))
    cst[:, _WO + 16:_WO + 32] = np.pad(Wo_c[128:256], ((0, 0), (0, 6)))
    cst[0, _ONES:_ONES + 128] = 1.0
    cst[0, _OC:_OC + 10] = oc
    # initial state vT0[h, b] = hhat0[h] for all b (chunked along h)
    cst[:, _VT0 + 0:_VT0 + 256] = hhat0[0:128, None]
    cst[:, _VT0 + 256:_VT0 + 512] = hhat0[128:256, None]
    return oh, cst


def prepare(x, W_embed, b_embed, W_update, b_update, gamma, beta, W_out,
            b_out, T_override=None, B_override=None):
    x = np.asarray(x, np.float32)
    B = x.shape[0] if B_override is None else B_override
    T = x.shape[1] if T_override is None else T_override
    x = x[:B, :T]

    oh, cst = _prepare_host(
        np.asarray(x), np.asarray(W_embed), np.asarray(b_embed),
        np.asarray(W_update), np.asarray(b_update), np.asarray(gamma),
        np.asarray(beta), np.asarray(W_out), np.asarray(b_out),
    )

    B_local = B // NCORES
    nc = build_nc(T, B_local)

    ohb = min(OHB, T)
    cst_b = _bf16(cst)
    in_maps = []
    for c in range(NCORES):
        sl = slice(c * B_local, (c + 1) * B_local)
        ohc = oh[:, :, sl]  # [T, NV, B_local]
        ohc = ohc.reshape(T // ohb, ohb, NV, B_local).transpose(0, 2, 1, 3)
        ohc = ohc.reshape(T // ohb, NV, ohb * B_local)
        in_maps.append({
            "oh": _bf16(np.ascontiguousarray(ohc)),
            "cst": cst_b,
        })
    return nc, in_maps


def _numpy_fallback(x, W_embed, b_embed, W_update, b_update, gamma, beta,
                    W_out, b_out):
    """Reference math on host; only for inputs the device kernel can't take
    (never happens with the spec'd randint fill, but better safe)."""
    xb = x[:, :, 0]
    B, T = xb.shape
    h = np.zeros((B, H), np.float32)
    for t in range(T):
        inp = np.tanh(xb[:, t:t + 1] @ W_embed + b_embed)
        z = (inp + h) @ W_update + b_update
        u = np.tanh(z)
        mu = u.mean(-1, keepdims=True)
        var = ((u - mu) ** 2).mean(-1, keepdims=True)
        h = (u - mu) / np.sqrt(var + EPS) * gamma + beta
    return (h @ W_out + b_out).astype(np.float32)


def kernel(x, W_embed, b_embed, W_update, b_update, gamma, beta, W_out,
           b_out, T_override=None, B_override=None):
    x = np.asarray(x, np.float32)
    xi = x[:, :, 0]
    if not (np.all(xi == np.round(xi)) and xi.min() >= 0 and xi.max() < NV
            and x.shape[0] % (NCORES * 256) == 0):
        return _numpy_fallback(
            x, np.asarray(W_embed, np.float32), np.asarray(b_embed, np.float32),
            np.asarray(W_update, np.float32), np.asarray(b_update, np.float32),
            np.asarray(gamma, np.float32), np.asarray(beta, np.float32),
            np.asarray(W_out, np.float32), np.asarray(b_out, np.float32))

    nc, in_maps = prepare(x, W_embed, b_embed, W_update, b_update, gamma,
                          beta, W_out, b_out, T_override, B_override)

    from concourse.bass_utils import run_bass_kernel_spmd

    res = run_bass_kernel_spmd(nc, in_maps, list(range(NCORES)))
    global LAST_RESULT
    LAST_RESULT = res
    outs = [np.asarray(res.results[c]["out"][:, :10], np.float32)
            for c in range(NCORES)]
    return np.concatenate(outs, axis=0)


LAST_RESULT = None
